# revision 1
# baseline (speedup 1.0000x reference)
"""CrossSparseGAT Trainium2 kernel (8 NeuronCores, SPMD).

Strategy (dst-sharded, edge blocks, gather-free):
  - Host: sort edges by dst, split dst space into 8 contiguous core ranges
    (~equal edge counts), greedy-pack dsts into blocks of <=128 dsts and
    <=KT*128 edges, pad each block's edge list to a multiple of 128.
    Per-edge node features are pre-expanded into a transposed channel
    xcatT[128, E_pad] = [src_feats[s]; dst_feats[d]] (bf16, data movement
    only - all model math stays on device).
  - Device, per 128-edge tile t:
      te = xcat_t.T @ Wcat2          (PE)  -> [V_e | A_src_e + A_dst_e]
        where Wcat2 = [[Wv.T, (W4@W2).T], [0, (W4@W1).T]]  (host-folded)
      l  = A_e + P*c + deter         (DVE, chunk-batched)
      expl = exp(leakyrelu(l)) * valid   (DVE+ACT)
      H  = onehot(dst_slot)          (DVE is_equal vs iota)
      agg_psum[block] += H.T @ [V_e * expl | expl]   (PE, PSUM-accumulated)
  - Block end: alpha-normalize by the segment sum, transpose, fused
    out/residual matmuls + bias + LayerNorm (batched across 8 blocks),
    store to a staged per-core output; host scatters staged rows to the
    final [N_DST, 64] array.
"""

import os
import sys
from contextlib import ExitStack
from dataclasses import dataclass, field

import numpy as np

for _p in ("/opt/trn_rl_repo", "/opt/pypackages"):
    if _p not in sys.path and os.path.isdir(_p):
        sys.path.append(_p)

import ml_dtypes

import concourse.bacc as bacc
import concourse.bass as bass
import concourse.tile as tile
from concourse import mybir
from concourse.masks import make_identity

F32 = mybir.dt.float32
BF16 = mybir.dt.bfloat16
I32 = mybir.dt.int32
AF = mybir.ActivationFunctionType
ALU = mybir.AluOpType

D = 64
NH = 8
HD = D // NH
C = D + NH  # 72
LN_EPS = 1e-5
PAD_DETER = -200.0


def _ap(t):
    return t if isinstance(t, bass.AP) else t[:]


def _mkap(base, dims, extra_offset=0):
    a = _ap(base)
    return bass.AP(tensor=a.tensor, offset=a.offset + extra_offset,
                   ap=[list(a.ap[0])] + [list(d) for d in dims])


@dataclass
class Cfg:
    n_cores: int
    n_dst: int
    n_src: int
    NB: int
    KT: int
    CHUNK_T: int
    G: int = 8
    TEG: int = 4  # tiles per te-psum group


@dataclass
class HostData:
    cfg: Cfg
    in_maps: list
    blocks: list
    meta: dict = field(default_factory=dict)


def _to_bf16(x):
    return np.asarray(x, dtype=ml_dtypes.bfloat16)


def prepare_host(inputs, n_cores=8, KT_candidates=(9, 10, 11, 12, 14, 18, 24)):
    dst_feats = np.asarray(inputs["dst_feats"], np.float32)
    src_feats = np.asarray(inputs["src_feats"], np.float32)
    edge_index = np.asarray(inputs["edge_index"], np.int32)
    P_edge = np.asarray(inputs["P_edge"], np.float32)
    deter_edge = np.asarray(inputs["deter_edge"], np.float32)
    W1 = np.asarray(inputs["W1"], np.float32)
    W2 = np.asarray(inputs["W2"], np.float32)
    W3 = np.asarray(inputs["W3"], np.float32)
    W4 = np.asarray(inputs["W4"], np.float32)
    Wv = np.asarray(inputs["Wv"], np.float32)
    Wout = np.asarray(inputs["Wout"], np.float32)
    b_out = np.asarray(inputs["b_out"], np.float32)
    Wres = np.asarray(inputs["Wres"], np.float32)
    b_res = np.asarray(inputs["b_res"], np.float32)
    ln_g = np.asarray(inputs["ln_g"], np.float32)
    ln_b = np.asarray(inputs["ln_b"], np.float32)

    n_dst = dst_feats.shape[0]
    n_src = src_feats.shape[0]
    E = edge_index.shape[1]

    src_idx = edge_index[0]
    dst_idx = edge_index[1]
    order = np.argsort(dst_idx, kind="stable")
    ds = dst_idx[order]
    ss = src_idx[order]
    Ps = P_edge[order]
    Des = deter_edge[order]

    counts = np.bincount(dst_idx, minlength=n_dst).astype(np.int64)
    cum = np.cumsum(counts)
    bounds = [0]
    for c in range(1, n_cores):
        t = np.searchsorted(cum, E * c // n_cores)
        bounds.append(min(int(t) + 1, n_dst))
    bounds.append(n_dst)
    for i in range(1, len(bounds)):
        bounds[i] = max(bounds[i], bounds[i - 1])

    dstart = np.concatenate([[0], cum]).astype(np.int64)

    def pack(core, KT):
        d0, d1 = bounds[core], bounds[core + 1]
        blocks = []
        d = d0
        cap = KT * 128
        while d < d1:
            e0 = dstart[d]
            nd = 0
            while d + nd < d1 and nd < 128:
                if counts[d + nd] > cap:
                    raise ValueError("dst too big")
                if dstart[d + nd + 1] - e0 > cap:
                    break
                nd += 1
            if nd == 0:
                raise ValueError("cannot place dst")
            blocks.append((int(d), int(nd)))
            d += nd
        return blocks

    best = None
    for KT in KT_candidates:
        try:
            bl = [pack(c, KT) for c in range(n_cores)]
        except ValueError:
            continue
        NB = max(len(b) for b in bl)
        cost = NB * KT
        if best is None or cost < best[0]:
            best = (cost, KT, NB, bl)
    assert best is not None, "no feasible KT"
    _, KT, NB, blocks_per_core = best

    NT = NB * KT
    cfg = Cfg(n_cores=n_cores, n_dst=n_dst, n_src=n_src, NB=NB, KT=KT,
              CHUNK_T=min(64, NT))

    # host-side constant folding of weights (tiny, weight-only)
    c8 = (W4 @ W3[:, 0]).astype(np.float32)
    wcat2 = np.zeros((2 * D, C), np.float32)
    wcat2[:D, :D] = Wv.T
    wcat2[:D, D:] = (W4 @ W2).T
    wcat2[D:, D:] = (W4 @ W1).T
    bias64 = (b_out + b_res).astype(np.float32)

    in_maps = []
    for c in range(n_cores):
        bl = blocks_per_core[c]
        g_src = np.zeros((NT, 128), np.int32)
        g_dstg = np.zeros((NT, 128), np.int32)
        g_slot = np.zeros((NT, 128), np.float32)
        g_P = np.zeros((NT, 128), np.float32)
        g_det = np.full((NT, 128), PAD_DETER, np.float32)
        g_val = np.zeros((NT, 128), np.float32)
        pad_mask = np.ones((NT, 128), bool)
        for b, (dst0, nd) in enumerate(bl):
            e0 = int(dstart[dst0])
            e1 = int(dstart[dst0 + nd]) if nd > 0 else e0
            ne = e1 - e0
            t0 = b * KT
            j = np.arange(ne)
            tt = t0 + j // 128
            pp = j % 128
            g_src[tt, pp] = ss[e0:e1]
            g_dstg[tt, pp] = ds[e0:e1]
            g_slot[tt, pp] = (ds[e0:e1] - dst0).astype(np.float32)
            g_P[tt, pp] = Ps[e0:e1]
            g_det[tt, pp] = Des[e0:e1]
            g_val[tt, pp] = 1.0
            pad_mask[tt, pp] = False

        # per-edge concatenated features, transposed: [128, NT*128] bf16
        xs = src_feats[g_src.ravel()]          # [NT*128, 64]
        xd = dst_feats[g_dstg.ravel()]         # [NT*128, 64]
        pm = pad_mask.ravel()
        xs[pm] = 0.0
        xd[pm] = 0.0
        xcatT = _to_bf16(np.concatenate([xs, xd], axis=1).T)  # [128, NT*128]
        xcatT = np.ascontiguousarray(xcatT)

        dslice = np.zeros((NB * 128, D), np.float32)
        for b, (dst0, nd) in enumerate(bl):
            if nd > 0:
                dslice[b * 128: b * 128 + nd] = dst_feats[dst0: dst0 + nd]

        m = {
            "dst_slice": dslice,
            "xcatT": xcatT,
            "ch_slot": _to_bf16(np.ascontiguousarray(g_slot.T)),
            "ch_P": np.ascontiguousarray(g_P.T),
            "ch_det": np.ascontiguousarray(g_det.T),
            "ch_val": _to_bf16(np.ascontiguousarray(g_val.T)),
            "wcat2": _to_bf16(wcat2),
            "wout_t": _to_bf16(Wout.T),
            "wres_t": _to_bf16(Wres.T),
            "c8": np.ascontiguousarray(c8.reshape(1, NH)),
            "bias64": np.ascontiguousarray(bias64.reshape(1, D)),
            "ln_g": np.ascontiguousarray(ln_g.reshape(1, D)),
            "ln_b": np.ascontiguousarray(ln_b.reshape(1, D)),
        }
        in_maps.append(m)

    return HostData(cfg=cfg, in_maps=in_maps, blocks=blocks_per_core)


def build_program(cfg: Cfg, debug=False):
    NB, KT, CT, G, TEG = cfg.NB, cfg.KT, cfg.CHUNK_T, cfg.G, cfg.TEG
    NT = NB * KT

    nc = bacc.Bacc("TRN2", target_bir_lowering=False, debug=debug,
                   num_devices=cfg.n_cores)

    dst_slice = nc.dram_tensor("dst_slice", [NB * 128, D], F32, kind="ExternalInput")
    xcatT_d = nc.dram_tensor("xcatT", [128, NT * 128], BF16, kind="ExternalInput")
    ch_slot = nc.dram_tensor("ch_slot", [128, NT], BF16, kind="ExternalInput")
    ch_P = nc.dram_tensor("ch_P", [128, NT], F32, kind="ExternalInput")
    ch_det = nc.dram_tensor("ch_det", [128, NT], F32, kind="ExternalInput")
    ch_val = nc.dram_tensor("ch_val", [128, NT], BF16, kind="ExternalInput")
    wcat2_d = nc.dram_tensor("wcat2", [2 * D, C], BF16, kind="ExternalInput")
    wout_t_d = nc.dram_tensor("wout_t", [D, D], BF16, kind="ExternalInput")
    wres_t_d = nc.dram_tensor("wres_t", [D, D], BF16, kind="ExternalInput")
    c8_d = nc.dram_tensor("c8", [1, NH], F32, kind="ExternalInput")
    bias64_d = nc.dram_tensor("bias64", [1, D], F32, kind="ExternalInput")
    ln_g_d = nc.dram_tensor("ln_g", [1, D], F32, kind="ExternalInput")
    ln_b_d = nc.dram_tensor("ln_b", [1, D], F32, kind="ExternalInput")

    staged = nc.dram_tensor("staged", [NB * 128, D], F32, kind="ExternalOutput")

    with tile.TileContext(nc) as tc, ExitStack() as ctx:
        consts = ctx.enter_context(tc.tile_pool(name="consts", bufs=1))
        pa = ctx.enter_context(tc.tile_pool(name="pa", bufs=3))
        pch = ctx.enter_context(tc.tile_pool(name="pch", bufs=2))
        pblk = ctx.enter_context(tc.tile_pool(name="pblk", bufs=3))
        pout = ctx.enter_context(tc.tile_pool(name="pout", bufs=2))
        psum = ctx.enter_context(tc.tile_pool(name="psum", bufs=2, space="PSUM"))

        ident_f = consts.tile([128, 128], F32, tag="ident_f")
        make_identity(nc, ident_f[:])
        ident_b = consts.tile([128, 128], BF16, tag="ident_b")
        make_identity(nc, ident_b[:])
        iota_i = consts.tile([128, 128], I32, tag="iota_i")
        nc.gpsimd.iota(iota_i[:], pattern=[[1, 128]], base=0, channel_multiplier=0)
        iota_rep = consts.tile([128, 128], BF16, tag="iota_rep")
        nc.vector.tensor_copy(iota_rep[:], iota_i[:])

        c_rep = consts.tile([128, NH], F32, tag="c_rep")
        nc.sync.dma_start(out=c_rep[:], in_=c8_d[:, :].to_broadcast([128, NH]))
        bias_rep = consts.tile([128, D], F32, tag="bias_rep")
        nc.sync.dma_start(out=bias_rep[:], in_=bias64_d[:, :].to_broadcast([128, D]))
        g_rep = consts.tile([128, D], F32, tag="g_rep")
        nc.sync.dma_start(out=g_rep[:], in_=ln_g_d[:, :].to_broadcast([128, D]))
        b_rep = consts.tile([128, D], F32, tag="b_rep")
        nc.sync.dma_start(out=b_rep[:], in_=ln_b_d[:, :].to_broadcast([128, D]))
        eps_t = consts.tile([128, 1], F32, tag="eps_t")
        nc.vector.memset(eps_t[:], LN_EPS)

        wcat2 = consts.tile([2 * D, C], BF16, tag="wcat2")
        nc.sync.dma_start(out=wcat2[:], in_=wcat2_d[:, :])
        wout_t = consts.tile([D, D], BF16, tag="wout_t")
        nc.sync.dma_start(out=wout_t[:], in_=wout_t_d[:, :])
        wres_t = consts.tile([D, D], BF16, tag="wres_t")
        nc.sync.dma_start(out=wres_t[:], in_=wres_t_d[:, :])

        xdstT = consts.tile([D, NB * 128], BF16, tag="xdstT")

        # ---- phase A: transpose dst features for the residual matmul ----
        PA_G = 4
        for t0 in range(0, NB, PA_G):
            ntl = min(PA_G, NB - t0)
            x4 = pa.tile([128, PA_G * D], F32, tag="x4")
            nc.sync.dma_start(
                out=x4[:, : ntl * D],
                in_=dst_slice[t0 * 128: (t0 + ntl) * 128, :].rearrange(
                    "(k p) d -> p k d", p=128),
            )
            xt_ps = psum.tile([D, PA_G * 128], F32, tag="tp", name=f"tpa{t0}")
            for k in range(ntl):
                nc.tensor.transpose(
                    out=xt_ps[:, k * 128: (k + 1) * 128],
                    in_=x4[:, k * D: (k + 1) * D],
                    identity=ident_f[:],
                )
            nc.scalar.copy(out=xdstT[:, t0 * 128: (t0 + ntl) * 128],
                           in_=xt_ps[:, : ntl * 128])

        # ---- phase B ----
        n_chunks = (NT + CT - 1) // CT
        blk_state = {}
        ln_group = []
        res_ps_holder = [None]

        def flush_ln(res_ps):
            g = len(ln_group)
            b0 = ln_group[0]
            W = g * D
            resb = pout.tile([128, G * D], F32, tag="resb")
            nc.vector.tensor_tensor(
                out=resb[:, :W], in0=res_ps[:, :W],
                in1=_mkap(bias_rep, [[0, g], [1, D]]), op=ALU.add)
            mean = pblk.tile([128, G], F32, tag="mean")
            nc.vector.tensor_reduce(
                out=mean[:, :g],
                in_=resb[:, :W].rearrange("p (g d) -> p g d", g=g),
                axis=mybir.AxisListType.X, op=ALU.add)
            nc.vector.tensor_scalar_mul(mean[:, :g], mean[:, :g], 1.0 / D)
            sq = pout.tile([128, G * D], F32, tag="sq")
            nc.gpsimd.tensor_tensor(sq[:, :W], resb[:, :W], resb[:, :W], op=ALU.mult)
            var = pblk.tile([128, G], F32, tag="var")
            nc.vector.tensor_reduce(
                out=var[:, :g],
                in_=sq[:, :W].rearrange("p (g d) -> p g d", g=g),
                axis=mybir.AxisListType.X, op=ALU.add)
            msq = pblk.tile([128, G], F32, tag="msq")
            nc.gpsimd.tensor_tensor(msq[:, :g], mean[:, :g], mean[:, :g], op=ALU.mult)
            nc.vector.scalar_tensor_tensor(
                out=var[:, :g], in0=var[:, :g], scalar=1.0 / D, in1=msq[:, :g],
                op0=ALU.mult, op1=ALU.subtract)
            std = pblk.tile([128, G], F32, tag="std")
            nc.scalar.activation(std[:, :g], var[:, :g], AF.Sqrt, bias=eps_t[:, :1])
            nc.vector.reciprocal(std[:, :g], std[:, :g])
            xm = pout.tile([128, G * D], F32, tag="xm")
            nc.vector.tensor_tensor(xm[:, :W], resb[:, :W],
                                    _mkap(mean, [[1, g], [0, D]]), op=ALU.subtract)
            nc.vector.tensor_tensor(xm[:, :W], xm[:, :W],
                                    _mkap(std, [[1, g], [0, D]]), op=ALU.mult)
            gb = pout.tile([128, G * D], F32, tag="gb")
            nc.vector.tensor_tensor(gb[:, :W], xm[:, :W],
                                    _mkap(g_rep, [[0, g], [1, D]]), op=ALU.mult)
            nc.vector.tensor_tensor(gb[:, :W], gb[:, :W],
                                    _mkap(b_rep, [[0, g], [1, D]]), op=ALU.add)
            nc.sync.dma_start(
                out=staged[b0 * 128: (b0 + g) * 128, :].rearrange(
                    "(g p) d -> p g d", p=128),
                in_=gb[:, :W])
            ln_group.clear()

        def block_end(b, agg_ps):
            gi = len(ln_group)
            if gi == 0:
                res_ps_holder[0] = psum.tile([128, G * D], F32, tag="res",
                                             name=f"res{b}")
            res_ps = res_ps_holder[0]
            ln_group.append(b)
            rcp = pblk.tile([128, NH], F32, tag="rcp")
            nc.vector.tensor_scalar_add(rcp[:], agg_ps[:, D:C], 1e-12)
            nc.vector.reciprocal(rcp[:], rcp[:])
            aggn = pblk.tile([128, D], BF16, tag="aggn")
            nc.vector.tensor_tensor(
                out=aggn[:], in0=agg_ps[:, :D],
                in1=_mkap(rcp, [[1, NH], [0, HD]]), op=ALU.mult)
            at_ps = psum.tile([D, 128], BF16, tag="tp", name=f"tpb{b}")
            nc.tensor.transpose(out=at_ps[:], in_=aggn[:], identity=ident_b[:])
            aggT = pblk.tile([D, 128], BF16, tag="aggT")
            nc.scalar.copy(out=aggT[:], in_=at_ps[:])
            nc.tensor.matmul(
                out=res_ps[:, gi * D: (gi + 1) * D],
                lhsT=aggT[:], rhs=wout_t[:], start=True, stop=False)
            nc.tensor.matmul(
                out=res_ps[:, gi * D: (gi + 1) * D],
                lhsT=xdstT[:, b * 128: (b + 1) * 128],
                rhs=wres_t[:], start=False, stop=True)
            if len(ln_group) == G or b == NB - 1:
                flush_ln(res_ps)

        for ci in range(n_chunks):
            t0 = ci * CT
            tl = min(CT, NT - t0)
            W8 = tl * NH

            xc = pch.tile([128, CT * 128], BF16, tag="xc")
            nc.sync.dma_start(xc[:, : tl * 128],
                              xcatT_d[:, t0 * 128: (t0 + tl) * 128])
            slot = pch.tile([128, CT], BF16, tag="slot")
            nc.sync.dma_start(slot[:, :tl], ch_slot[:, t0: t0 + tl])
            Pt = pch.tile([128, CT], F32, tag="Pt")
            nc.sync.dma_start(Pt[:, :tl], ch_P[:, t0: t0 + tl])
            det = pch.tile([128, CT], F32, tag="det")
            nc.sync.dma_start(det[:, :tl], ch_det[:, t0: t0 + tl])
            val = pch.tile([128, CT], BF16, tag="val")
            nc.sync.dma_start(val[:, :tl], ch_val[:, t0: t0 + tl])

            # per-edge projections te = xcat.T @ wcat2 -> [V | A]
            te_sb = pch.tile([128, CT * C], BF16, tag="te")
            ng = (tl + TEG - 1) // TEG
            for g0 in range(ng):
                k0 = g0 * TEG
                kn = min(TEG, tl - k0)
                te_ps = psum.tile([128, TEG * C], F32, tag="tep",
                                  name=f"tep{ci}_{g0}")
                for kk in range(kn):
                    j = k0 + kk
                    nc.tensor.matmul(
                        out=te_ps[:, kk * C: (kk + 1) * C],
                        lhsT=xc[:, j * 128: (j + 1) * 128],
                        rhs=wcat2[:], start=True, stop=True)
                cp = te_sb[:, k0 * C: (k0 + kn) * C]
                nc.scalar.copy(out=cp, in_=te_ps[:, : kn * C])

            # l = P*c + det + A
            lbuf = pch.tile([128, CT * NH], F32, tag="lbuf")
            nc.vector.tensor_tensor(
                out=lbuf[:, :W8],
                in0=_mkap(Pt, [[1, tl], [0, NH]]),
                in1=_mkap(c_rep, [[0, tl], [1, NH]]), op=ALU.mult)
            nc.vector.tensor_tensor(lbuf[:, :W8], lbuf[:, :W8],
                                    _mkap(det, [[1, tl], [0, NH]]), op=ALU.add)
            nc.vector.tensor_tensor(lbuf[:, :W8], lbuf[:, :W8],
                                    _mkap(te_sb, [[C, tl], [1, NH]], extra_offset=D),
                                    op=ALU.add)
            nc.vector.scalar_tensor_tensor(
                out=lbuf[:, :W8], in0=lbuf[:, :W8], scalar=0.2, in1=lbuf[:, :W8],
                op0=ALU.mult, op1=ALU.max)
            rhs_buf = pch.tile([128, CT * C], BF16, tag="rhs")
            expl_view = _mkap(rhs_buf, [[C, tl], [1, NH]], extra_offset=D)
            nc.scalar.activation(expl_view, lbuf[:, :W8], AF.Exp)
            nc.vector.tensor_tensor(
                out=_mkap(rhs_buf, [[C, tl], [1, D]]),
                in0=_mkap(te_sb, [[C, tl], [1, D]]),
                in1=_mkap(rhs_buf, [[C, tl], [1, NH], [0, HD]], extra_offset=D),
                op=ALU.mult)
            H_all = pch.tile([128, CT * 128], BF16, tag="H")
            nc.vector.tensor_tensor(
                out=H_all[:, : tl * 128],
                in0=_mkap(slot, [[1, tl], [0, 128]]),
                in1=_mkap(iota_rep, [[0, tl], [1, 128]]),
                op=ALU.is_equal)
            for j in range(tl):
                t = t0 + j
                b, k = divmod(t, KT)
                if k == 0:
                    blk_state[b] = psum.tile([128, C], F32, tag="agg",
                                             name=f"agg{b}")
                nc.tensor.matmul(
                    out=blk_state[b][:],
                    lhsT=H_all[:, j * 128: (j + 1) * 128],
                    rhs=rhs_buf[:, j * C: (j + 1) * C],
                    start=(k == 0), stop=(k == KT - 1))
                if k == KT - 1:
                    block_end(b, blk_state.pop(b))

        assert not ln_group, "unflushed LN group"

    nc.compile()
    return nc


def unpack_output(hd: HostData, results):
    cfg = hd.cfg
    out = np.zeros((cfg.n_dst, D), np.float32)
    for c in range(cfg.n_cores):
        staged = results[c]["staged"]
        for b, (dst0, nd) in enumerate(hd.blocks[c]):
            if nd > 0:
                out[dst0: dst0 + nd] = staged[b * 128: b * 128 + nd]
    return out


_prog_cache = {}


def kernel(**inputs) -> np.ndarray:
    from concourse.bass_utils import run_bass_kernel_spmd

    hd = prepare_host(inputs, n_cores=8)
    key = (hd.cfg.NB, hd.cfg.KT, hd.cfg.CHUNK_T)
    if key not in _prog_cache:
        _prog_cache[key] = build_program(hd.cfg)
    nc = _prog_cache[key]
    res = run_bass_kernel_spmd(
        nc, hd.in_maps, core_ids=list(range(hd.cfg.n_cores)),
        trace=bool(int(os.environ.get("KERNEL_TRACE", "0"))),
    )
    out = unpack_output(hd, res.results)
    if res.exec_time_ns is not None:
        print(f"HW exec time: {res.exec_time_ns} ns", file=sys.stderr)
        kernel.last_exec_time_ns = res.exec_time_ns
    return out


kernel.last_exec_time_ns = None



# revision 3
# speedup vs baseline: 1.4372x; 1.4372x over previous
"""CrossSparseGAT Trainium2 kernel (8 NeuronCores, SPMD).

Strategy (dst-sharded, edge blocks, gather-free), v2:
  - Host: sort edges by dst, split dst space into 8 contiguous core ranges
    (~equal edge counts), greedy-pack dsts into blocks of <=128 dsts and
    <=KT*128 edges, pad each block's edge list to a multiple of 128.
    Per-edge concatenated features shipped transposed as
    xcatT[128, E_pad] = [src_feats[s]; dst_feats[d]] (bf16, data movement
    only - all model math stays on device). slot/P/deter shipped as
    column-duplicated bf16 pairs so every big DVE op qualifies for the
    2x (2-byte contiguous) fast path.
  - Device, per 128-edge tile t:
      te = xcat_t.T @ wcat2p         (PE)  -> [Vperm | A]  (V cols (hd,h)-major)
      l  = P*c + det + A; leaky      (DVE, bf16, chunk-batched, 2x)
      expl = exp(l)                  (ACT) -> rhs[:,64:72]
      rhs[:,0:64] = V * expl_bcast   (DVE, 2x via (hd,h) V layout)
      H  = onehot(dst_slot)          (DVE is_equal, 2x via slot-dup pairs)
      agg_psum[grp][blk] += H.T @ rhs (PE, PSUM-accumulated, 4 blocks/group)
  - Group end (4 blocks): batched alpha-normalize, PE transposes into a
    [aggT; xdstT] concat tile, ONE matmul per block vs stacked
    [Wout;Wres], batched LayerNorm, store staged rows; host scatters.
"""

import os
import sys
from contextlib import ExitStack
from dataclasses import dataclass, field

import numpy as np

for _p in ("/opt/trn_rl_repo", "/opt/pypackages"):
    if _p not in sys.path and os.path.isdir(_p):
        sys.path.append(_p)

import ml_dtypes

import concourse.bacc as bacc
import concourse.bass as bass
import concourse.tile as tile
from concourse import mybir
from concourse.masks import make_identity

F32 = mybir.dt.float32
BF16 = mybir.dt.bfloat16
I32 = mybir.dt.int32
AF = mybir.ActivationFunctionType
ALU = mybir.AluOpType

D = 64
NH = 8
HD = D // NH
C = D + NH  # 72
LN_EPS = 1e-5
PAD_DETER = -200.0
G = 4       # blocks per norm/res/LN group
TEG = 7     # tiles per te-psum group (7*72 = 504 <= 512 f32 bank)
CT = 63     # chunk tiles (multiple of TEG)

# (hd, h)-major permutation of the 64 V channels
VPERM = np.array([(j % 8) * 8 + j // 8 for j in range(D)], np.int64)


def _ap(t):
    return t if isinstance(t, bass.AP) else t[:]


def _mkap(base, dims, extra_offset=0):
    a = _ap(base)
    return bass.AP(tensor=a.tensor, offset=a.offset + extra_offset,
                   ap=[list(a.ap[0])] + [list(d) for d in dims])


@dataclass
class Cfg:
    n_cores: int
    n_dst: int
    n_src: int
    NB: int
    KT: int


@dataclass
class HostData:
    cfg: Cfg
    in_maps: list
    blocks: list
    meta: dict = field(default_factory=dict)


def _to_bf16(x):
    return np.asarray(x, dtype=ml_dtypes.bfloat16)


def prepare_host(inputs, n_cores=8, KT_candidates=(9, 10, 11, 12, 14, 18, 24)):
    dst_feats = np.asarray(inputs["dst_feats"], np.float32)
    src_feats = np.asarray(inputs["src_feats"], np.float32)
    edge_index = np.asarray(inputs["edge_index"], np.int32)
    P_edge = np.asarray(inputs["P_edge"], np.float32)
    deter_edge = np.asarray(inputs["deter_edge"], np.float32)
    W1 = np.asarray(inputs["W1"], np.float32)
    W2 = np.asarray(inputs["W2"], np.float32)
    W3 = np.asarray(inputs["W3"], np.float32)
    W4 = np.asarray(inputs["W4"], np.float32)
    Wv = np.asarray(inputs["Wv"], np.float32)
    Wout = np.asarray(inputs["Wout"], np.float32)
    b_out = np.asarray(inputs["b_out"], np.float32)
    Wres = np.asarray(inputs["Wres"], np.float32)
    b_res = np.asarray(inputs["b_res"], np.float32)
    ln_g = np.asarray(inputs["ln_g"], np.float32)
    ln_b = np.asarray(inputs["ln_b"], np.float32)

    n_dst = dst_feats.shape[0]
    n_src = src_feats.shape[0]
    E = edge_index.shape[1]

    src_idx = edge_index[0]
    dst_idx = edge_index[1]
    order = np.argsort(dst_idx, kind="stable")
    ds = dst_idx[order]
    ss = src_idx[order]
    Ps = P_edge[order]
    Des = deter_edge[order]

    counts = np.bincount(dst_idx, minlength=n_dst).astype(np.int64)
    cum = np.cumsum(counts)
    bounds = [0]
    for c in range(1, n_cores):
        t = np.searchsorted(cum, E * c // n_cores)
        bounds.append(min(int(t) + 1, n_dst))
    bounds.append(n_dst)
    for i in range(1, len(bounds)):
        bounds[i] = max(bounds[i], bounds[i - 1])

    dstart = np.concatenate([[0], cum]).astype(np.int64)

    def pack(core, KT):
        d0, d1 = bounds[core], bounds[core + 1]
        blocks = []
        d = d0
        cap = KT * 128
        while d < d1:
            e0 = dstart[d]
            nd = 0
            while d + nd < d1 and nd < 128:
                if counts[d + nd] > cap:
                    raise ValueError("dst too big")
                if dstart[d + nd + 1] - e0 > cap:
                    break
                nd += 1
            if nd == 0:
                raise ValueError("cannot place dst")
            blocks.append((int(d), int(nd)))
            d += nd
        return blocks

    best = None
    for KT in KT_candidates:
        try:
            bl = [pack(c, KT) for c in range(n_cores)]
        except ValueError:
            continue
        NB = max(len(b) for b in bl)
        cost = NB * KT
        if best is None or cost < best[0]:
            best = (cost, KT, NB, bl)
    assert best is not None, "no feasible KT"
    _, KT, NB, blocks_per_core = best

    NT = NB * KT
    cfg = Cfg(n_cores=n_cores, n_dst=n_dst, n_src=n_src, NB=NB, KT=KT)

    # host-side constant folding of weights (tiny, weight-only)
    c8 = (W4 @ W3[:, 0]).astype(np.float32)
    wcat2 = np.zeros((2 * D, C), np.float32)
    wcat2[:D, :D] = Wv.T[:, VPERM]
    wcat2[:D, D:] = (W4 @ W2).T
    wcat2[D:, D:] = (W4 @ W1).T
    wor = np.concatenate([Wout.T[VPERM, :], Wres.T], axis=0)  # [128, 64]
    bias64 = (b_out + b_res).astype(np.float32)
    iota128 = np.broadcast_to(np.arange(128, dtype=np.float32)[None, :],
                              (128, 128))

    in_maps = []
    for c in range(n_cores):
        bl = blocks_per_core[c]
        g_src = np.zeros((NT, 128), np.int32)
        g_dstg = np.zeros((NT, 128), np.int32)
        g_slot = np.zeros((NT, 128), np.float32)
        g_P = np.zeros((NT, 128), np.float32)
        g_det = np.full((NT, 128), PAD_DETER, np.float32)
        pad_mask = np.ones((NT, 128), bool)
        for b, (dst0, nd) in enumerate(bl):
            e0 = int(dstart[dst0])
            e1 = int(dstart[dst0 + nd]) if nd > 0 else e0
            ne = e1 - e0
            t0 = b * KT
            j = np.arange(ne)
            tt = t0 + j // 128
            pp = j % 128
            g_src[tt, pp] = ss[e0:e1]
            g_dstg[tt, pp] = ds[e0:e1]
            g_slot[tt, pp] = (ds[e0:e1] - dst0).astype(np.float32)
            g_P[tt, pp] = Ps[e0:e1]
            g_det[tt, pp] = Des[e0:e1]
            pad_mask[tt, pp] = False

        # per-edge concatenated features, transposed: [128, NT*128] bf16
        xs = src_feats[g_src.ravel()]          # [NT*128, 64]
        xd = dst_feats[g_dstg.ravel()]         # [NT*128, 64]
        pm = pad_mask.ravel()
        xs[pm] = 0.0
        xd[pm] = 0.0
        xcatT = _to_bf16(np.concatenate([xs, xd], axis=1).T)  # [128, NT*128]
        xcatT = np.ascontiguousarray(xcatT)

        # slot/P/det as column-duplicated bf16: spd[:, t*6 + {0,1;2,3;4,5}]
        spd = np.empty((128, NT, 6), np.float32)
        spd[:, :, 0] = spd[:, :, 1] = g_slot.T
        spd[:, :, 2] = spd[:, :, 3] = g_P.T
        spd[:, :, 4] = spd[:, :, 5] = g_det.T
        spd = _to_bf16(spd.reshape(128, NT * 6))

        # per-block dst features, transposed: [64, NB*128]
        dslice = np.zeros((NB * 128, D), np.float32)
        for b, (dst0, nd) in enumerate(bl):
            if nd > 0:
                dslice[b * 128: b * 128 + nd] = dst_feats[dst0: dst0 + nd]
        xdstT = _to_bf16(np.ascontiguousarray(dslice.T))

        m = {
            "xcatT": xcatT,
            "spd": np.ascontiguousarray(spd),
            "xdstT": xdstT,
            "iota128": _to_bf16(iota128),
            "wcat2": _to_bf16(wcat2),
            "wor": _to_bf16(wor),
            "c8": _to_bf16(c8.reshape(1, NH)),
            "bias64": np.ascontiguousarray(bias64.reshape(1, D)),
            "ln_g": np.ascontiguousarray(ln_g.reshape(1, D)),
            "ln_b": np.ascontiguousarray(ln_b.reshape(1, D)),
        }
        in_maps.append(m)

    return HostData(cfg=cfg, in_maps=in_maps, blocks=blocks_per_core)


def build_program(cfg: Cfg, debug=False):
    NB, KT = cfg.NB, cfg.KT
    NT = NB * KT

    nc = bacc.Bacc("TRN2", target_bir_lowering=False, debug=debug,
                   num_devices=cfg.n_cores)

    xcatT_d = nc.dram_tensor("xcatT", [128, NT * 128], BF16, kind="ExternalInput")
    spd_d = nc.dram_tensor("spd", [128, NT * 6], BF16, kind="ExternalInput")
    xdstT_d = nc.dram_tensor("xdstT", [D, NB * 128], BF16, kind="ExternalInput")
    iota_d = nc.dram_tensor("iota128", [128, 128], BF16, kind="ExternalInput")
    wcat2_d = nc.dram_tensor("wcat2", [2 * D, C], BF16, kind="ExternalInput")
    wor_d = nc.dram_tensor("wor", [2 * D, D], BF16, kind="ExternalInput")
    c8_d = nc.dram_tensor("c8", [1, NH], BF16, kind="ExternalInput")
    bias64_d = nc.dram_tensor("bias64", [1, D], F32, kind="ExternalInput")
    ln_g_d = nc.dram_tensor("ln_g", [1, D], F32, kind="ExternalInput")
    ln_b_d = nc.dram_tensor("ln_b", [1, D], F32, kind="ExternalInput")

    staged = nc.dram_tensor("staged", [NB * 128, D], F32, kind="ExternalOutput")

    with tile.TileContext(nc) as tc, ExitStack() as ctx:
        consts = ctx.enter_context(tc.tile_pool(name="consts", bufs=1))
        pch = ctx.enter_context(tc.tile_pool(name="pch", bufs=2))
        pblk = ctx.enter_context(tc.tile_pool(name="pblk", bufs=2))
        pout = ctx.enter_context(tc.tile_pool(name="pout", bufs=2))
        psum2 = ctx.enter_context(tc.tile_pool(name="psum2", bufs=2, space="PSUM"))
        psum1 = ctx.enter_context(tc.tile_pool(name="psum1", bufs=1, space="PSUM"))

        ident_b = consts.tile([128, 128], BF16, tag="ident_b")
        make_identity(nc, ident_b[:])
        iota_b = consts.tile([128, 128], BF16, tag="iota_b")
        nc.sync.dma_start(out=iota_b[:], in_=iota_d[:, :])

        c_rep = consts.tile([128, NH], BF16, tag="c_rep")
        nc.sync.dma_start(out=c_rep[:], in_=c8_d[:, :].to_broadcast([128, NH]))
        bias_rep = consts.tile([128, D], F32, tag="bias_rep")
        nc.sync.dma_start(out=bias_rep[:], in_=bias64_d[:, :].to_broadcast([128, D]))
        g_rep = consts.tile([128, D], F32, tag="g_rep")
        nc.sync.dma_start(out=g_rep[:], in_=ln_g_d[:, :].to_broadcast([128, D]))
        b_rep = consts.tile([128, D], F32, tag="b_rep")
        nc.sync.dma_start(out=b_rep[:], in_=ln_b_d[:, :].to_broadcast([128, D]))
        eps_t = consts.tile([128, 1], F32, tag="eps_t")
        nc.vector.memset(eps_t[:], LN_EPS)

        wcat2 = consts.tile([2 * D, C], BF16, tag="wcat2")
        nc.sync.dma_start(out=wcat2[:], in_=wcat2_d[:, :])
        wor_t = consts.tile([2 * D, D], BF16, tag="wor_t")
        nc.sync.dma_start(out=wor_t[:], in_=wor_d[:, :])

        # concat tile: top half gets per-group aggT, bottom half xdstT (static)
        catXD = consts.tile([128, NB * 128], BF16, tag="catXD")
        nc.sync.dma_start(out=catXD[D:2 * D, :], in_=xdstT_d[:, :])

        n_chunks = (NT + CT - 1) // CT
        agg_state = {}

        def flush_ln(g0b, res_ps, W):
            # res_ps: [128, 512] f32, block i at cols i*128 .. i*128+64
            WD = W * D
            resb = pout.tile([128, G * D], F32, tag="resb")
            nc.vector.tensor_tensor(
                out=resb[:, :WD],
                in0=_mkap(res_ps, [[128, W], [1, D]]),
                in1=_mkap(bias_rep, [[0, W], [1, D]]), op=ALU.add)
            mean = pblk.tile([128, G], F32, tag="mean")
            nc.vector.tensor_reduce(
                out=mean[:, :W],
                in_=resb[:, :WD].rearrange("p (g d) -> p g d", g=W),
                axis=mybir.AxisListType.X, op=ALU.add)
            nc.vector.tensor_scalar_mul(mean[:, :W], mean[:, :W], 1.0 / D)
            sq = pout.tile([128, G * D], F32, tag="sq")
            nc.gpsimd.tensor_tensor(sq[:, :WD], resb[:, :WD], resb[:, :WD],
                                    op=ALU.mult)
            var = pblk.tile([128, G], F32, tag="var")
            nc.vector.tensor_reduce(
                out=var[:, :W],
                in_=sq[:, :WD].rearrange("p (g d) -> p g d", g=W),
                axis=mybir.AxisListType.X, op=ALU.add)
            msq = pblk.tile([128, G], F32, tag="msq")
            nc.gpsimd.tensor_tensor(msq[:, :W], mean[:, :W], mean[:, :W],
                                    op=ALU.mult)
            nc.vector.scalar_tensor_tensor(
                out=var[:, :W], in0=var[:, :W], scalar=1.0 / D, in1=msq[:, :W],
                op0=ALU.mult, op1=ALU.subtract)
            std = pblk.tile([128, G], F32, tag="std")
            nc.scalar.activation(std[:, :W], var[:, :W], AF.Sqrt,
                                 bias=eps_t[:, :1])
            nc.vector.reciprocal(std[:, :W], std[:, :W])
            xm = pout.tile([128, G * D], F32, tag="xm")
            nc.vector.tensor_tensor(xm[:, :WD], resb[:, :WD],
                                    _mkap(mean, [[1, W], [0, D]]),
                                    op=ALU.subtract)
            nc.vector.tensor_tensor(xm[:, :WD], xm[:, :WD],
                                    _mkap(std, [[1, W], [0, D]]), op=ALU.mult)
            gb = pout.tile([128, G * D], F32, tag="gb")
            nc.gpsimd.tensor_tensor(gb[:, :WD], xm[:, :WD],
                                    _mkap(g_rep, [[0, W], [1, D]]), op=ALU.mult)
            nc.vector.tensor_tensor(gb[:, :WD], gb[:, :WD],
                                    _mkap(b_rep, [[0, W], [1, D]]), op=ALU.add)
            nc.sync.dma_start(
                out=staged[g0b * 128: (g0b + W) * 128, :].rearrange(
                    "(g p) d -> p g d", p=128),
                in_=gb[:, :WD])

        def group_end(g, agg_ps, W):
            g0b = g * G
            # per-head sums -> reciprocal (f32)
            rcs = pblk.tile([128, G * NH], F32, tag="rcs")
            nc.vector.tensor_scalar_add(
                rcs[:, :W * NH],
                _mkap(agg_ps, [[128, W], [1, NH]], extra_offset=D), 1e-12)
            nc.vector.reciprocal(rcs[:, :W * NH], rcs[:, :W * NH])
            # normalized agg (bf16), V channels are (hd,h)-major so the
            # divisor broadcast has a contiguous inner dim
            aggn = pblk.tile([128, G * D], BF16, tag="aggn")
            nc.vector.tensor_tensor(
                out=_mkap(aggn, [[D, W], [NH, NH], [1, NH]]),
                in0=_mkap(agg_ps, [[128, W], [NH, NH], [1, NH]]),
                in1=_mkap(rcs, [[NH, W], [0, NH], [1, NH]]),
                op=ALU.mult)
            # transpose each block into the concat tile's top half
            at_ps = psum1.tile([D, G * 128], BF16, tag="at", name=f"at{g}")
            for i in range(W):
                nc.tensor.transpose(
                    out=at_ps[:, i * 128: (i + 1) * 128],
                    in_=aggn[:, i * D: (i + 1) * D],
                    identity=ident_b[:])
            nc.vector.tensor_copy(
                catXD[:D, g0b * 128: (g0b + W) * 128], at_ps[:, : W * 128])
            # res = [aggT; xdstT].T @ [Wout_p; Wres]  (one matmul per block)
            res_ps = psum2.tile([128, 512], F32, tag="res", name=f"res{g}")
            for i in range(W):
                nc.tensor.matmul(
                    out=res_ps[:, i * 128: i * 128 + D],
                    lhsT=catXD[:, (g0b + i) * 128: (g0b + i + 1) * 128],
                    rhs=wor_t[:], start=True, stop=True)
            flush_ln(g0b, res_ps, W)

        for ci in range(n_chunks):
            t0 = ci * CT
            tl = min(CT, NT - t0)

            xc = pch.tile([128, CT * 128], BF16, tag="xc")
            nc.sync.dma_start(xc[:, : tl * 128],
                              xcatT_d[:, t0 * 128: (t0 + tl) * 128])
            spd = pch.tile([128, CT * 6], BF16, tag="spd")
            nc.sync.dma_start(spd[:, : tl * 6],
                              spd_d[:, t0 * 6: (t0 + tl) * 6])

            teV_sb = pch.tile([128, CT * D], BF16, tag="teV")
            A_sb = pch.tile([128, CT * NH], BF16, tag="A")

            ng = (tl + TEG - 1) // TEG
            for g0 in range(ng):
                k0 = g0 * TEG
                kn = min(TEG, tl - k0)
                te_ps = psum2.tile([128, 512], F32, tag="te",
                                   name=f"te{ci}_{g0}")
                for kk in range(kn):
                    j = k0 + kk
                    nc.tensor.matmul(
                        out=te_ps[:, kk * C: (kk + 1) * C],
                        lhsT=xc[:, j * 128: (j + 1) * 128],
                        rhs=wcat2[:], start=True, stop=True)
                nc.scalar.copy(
                    out=_mkap(teV_sb, [[D, kn], [1, D]], extra_offset=k0 * D),
                    in_=_mkap(te_ps, [[C, kn], [1, D]]))
                nc.scalar.copy(
                    out=_mkap(A_sb, [[NH, kn], [1, NH]], extra_offset=k0 * NH),
                    in_=_mkap(te_ps, [[C, kn], [1, NH]], extra_offset=D))

            # logits: l = P*c8 + det + A ; leaky(0.2)  (all bf16, 2x eligible)
            lbuf = pch.tile([128, CT * NH], BF16, tag="lbuf")
            nc.vector.tensor_tensor(
                out=_mkap(lbuf, [[NH, tl], [2, 4], [1, 2]]),
                in0=_mkap(spd, [[6, tl], [0, 4], [1, 2]], extra_offset=2),
                in1=_mkap(c_rep, [[0, tl], [2, 4], [1, 2]]), op=ALU.mult)
            nc.vector.tensor_tensor(
                out=_mkap(lbuf, [[NH, tl], [2, 4], [1, 2]]),
                in0=_mkap(lbuf, [[NH, tl], [2, 4], [1, 2]]),
                in1=_mkap(spd, [[6, tl], [0, 4], [1, 2]], extra_offset=4),
                op=ALU.add)
            nc.vector.tensor_tensor(
                out=_mkap(lbuf, [[NH, tl], [2, 4], [1, 2]]),
                in0=_mkap(lbuf, [[NH, tl], [2, 4], [1, 2]]),
                in1=_mkap(A_sb, [[NH, tl], [2, 4], [1, 2]]), op=ALU.add)
            nc.vector.scalar_tensor_tensor(
                out=lbuf[:, : tl * NH], in0=lbuf[:, : tl * NH], scalar=0.2,
                in1=lbuf[:, : tl * NH], op0=ALU.mult, op1=ALU.max)

            rhs_buf = pch.tile([128, CT * C], BF16, tag="rhs")
            nc.scalar.activation(
                _mkap(rhs_buf, [[C, tl], [1, NH]], extra_offset=D),
                lbuf[:, : tl * NH], AF.Exp)
            # V * expl (V channels (hd,h)-major -> contiguous bcast inner dim)
            nc.vector.tensor_tensor(
                out=_mkap(rhs_buf, [[C, tl], [NH, NH], [1, NH]]),
                in0=_mkap(teV_sb, [[D, tl], [NH, NH], [1, NH]]),
                in1=_mkap(rhs_buf, [[C, tl], [0, NH], [1, NH]],
                          extra_offset=D),
                op=ALU.mult)

            # one-hot H via slot-dup pairs (2x eligible)
            H_all = pch.tile([128, CT * 128], BF16, tag="H")
            nc.vector.tensor_tensor(
                out=_mkap(H_all, [[128, tl], [2, 64], [1, 2]]),
                in0=_mkap(spd, [[6, tl], [0, 64], [1, 2]]),
                in1=_mkap(iota_b, [[0, tl], [2, 64], [1, 2]]),
                op=ALU.is_equal)

            for j in range(tl):
                t = t0 + j
                b, k = divmod(t, KT)
                g, bi = divmod(b, G)
                if k == 0 and bi == 0:
                    agg_state[g] = psum2.tile([128, 512], F32, tag="agg",
                                              name=f"agg{g}")
                nc.tensor.matmul(
                    out=agg_state[g][:, bi * 128: bi * 128 + C],
                    lhsT=H_all[:, j * 128: (j + 1) * 128],
                    rhs=rhs_buf[:, j * C: (j + 1) * C],
                    start=(k == 0), stop=(k == KT - 1))
                if k == KT - 1 and (bi == G - 1 or b == NB - 1):
                    group_end(g, agg_state.pop(g), bi + 1)

        assert not agg_state, "unflushed agg group"

    nc.compile()
    return nc


def unpack_output(hd: HostData, results):
    cfg = hd.cfg
    out = np.zeros((cfg.n_dst, D), np.float32)
    for c in range(cfg.n_cores):
        staged = results[c]["staged"]
        for b, (dst0, nd) in enumerate(hd.blocks[c]):
            if nd > 0:
                out[dst0: dst0 + nd] = staged[b * 128: b * 128 + nd]
    return out


_prog_cache = {}


def kernel(**inputs) -> np.ndarray:
    from concourse.bass_utils import run_bass_kernel_spmd

    hd = prepare_host(inputs, n_cores=8)
    key = (hd.cfg.NB, hd.cfg.KT)
    if key not in _prog_cache:
        _prog_cache[key] = build_program(hd.cfg)
    nc = _prog_cache[key]
    res = run_bass_kernel_spmd(
        nc, hd.in_maps, core_ids=list(range(hd.cfg.n_cores)),
        trace=bool(int(os.environ.get("KERNEL_TRACE", "0"))),
    )
    out = unpack_output(hd, res.results)
    if res.exec_time_ns is not None:
        print(f"HW exec time: {res.exec_time_ns} ns", file=sys.stderr)
        kernel.last_exec_time_ns = res.exec_time_ns
    return out


kernel.last_exec_time_ns = None


# revision 10
# speedup vs baseline: 1.5059x; 1.0478x over previous
"""CrossSparseGAT Trainium2 kernel (8 NeuronCores, SPMD).

Strategy (dst-sharded, edge blocks, gather-free), v2:
  - Host: sort edges by dst, split dst space into 8 contiguous core ranges
    (~equal edge counts), greedy-pack dsts into blocks of <=128 dsts and
    <=KT*128 edges, pad each block's edge list to a multiple of 128.
    Per-edge concatenated features shipped transposed as
    xcatT[128, E_pad] = [src_feats[s]; dst_feats[d]] (bf16, data movement
    only - all model math stays on device). slot/P/deter shipped as
    column-duplicated bf16 pairs so every big DVE op qualifies for the
    2x (2-byte contiguous) fast path.
  - Device, per 128-edge tile t:
      te = xcat_t.T @ wcat2p         (PE)  -> [Vperm | A]  (V cols (hd,h)-major)
      l  = P*c + det + A; leaky      (DVE, bf16, chunk-batched, 2x)
      expl = exp(l)                  (ACT) -> rhs[:,64:72]
      rhs[:,0:64] = V * expl_bcast   (DVE, 2x via (hd,h) V layout)
      H  = onehot(dst_slot)          (DVE is_equal, 2x via slot-dup pairs)
      agg_psum[grp][blk] += H.T @ rhs (PE, PSUM-accumulated, 4 blocks/group)
  - Group end (4 blocks): batched alpha-normalize, PE transposes into a
    [aggT; xdstT] concat tile, ONE matmul per block vs stacked
    [Wout;Wres], batched LayerNorm, store staged rows; host scatters.
"""

import os
import sys
from contextlib import ExitStack
from dataclasses import dataclass, field

import numpy as np

for _p in ("/opt/trn_rl_repo", "/opt/pypackages"):
    if _p not in sys.path and os.path.isdir(_p):
        sys.path.append(_p)

import ml_dtypes

import concourse.bacc as bacc
import concourse.bass as bass
import concourse.tile as tile
from concourse import mybir
from concourse.masks import make_identity

F32 = mybir.dt.float32
BF16 = mybir.dt.bfloat16
I32 = mybir.dt.int32
AF = mybir.ActivationFunctionType
ALU = mybir.AluOpType

D = 64
NH = 8
HD = D // NH
C = D + NH  # 72
LN_EPS = 1e-5
PAD_DETER = -200.0
G = 8       # blocks per norm/res group (agg psum = 2 banks)
GLN = 16    # blocks per LayerNorm flush (2 groups)
TEG = 7     # tiles per te-psum group (7*72 = 504 <= 512 f32 bank)
CT = 63     # chunk tiles (multiple of TEG)

# (hd, h)-major permutation of the 64 V channels
VPERM = np.array([(j % 8) * 8 + j // 8 for j in range(D)], np.int64)


def _ap(t):
    return t if isinstance(t, bass.AP) else t[:]


def _mkap(base, dims, extra_offset=0):
    a = _ap(base)
    return bass.AP(tensor=a.tensor, offset=a.offset + extra_offset,
                   ap=[list(a.ap[0])] + [list(d) for d in dims])


@dataclass
class Cfg:
    n_cores: int
    n_dst: int
    n_src: int
    NB: int
    KT: int


@dataclass
class HostData:
    cfg: Cfg
    in_maps: list
    blocks: list
    meta: dict = field(default_factory=dict)


def _to_bf16(x):
    return np.asarray(x, dtype=ml_dtypes.bfloat16)


def prepare_host(inputs, n_cores=8, KT_candidates=(9, 10, 11, 12, 14, 18, 24)):
    dst_feats = np.asarray(inputs["dst_feats"], np.float32)
    src_feats = np.asarray(inputs["src_feats"], np.float32)
    edge_index = np.asarray(inputs["edge_index"], np.int32)
    P_edge = np.asarray(inputs["P_edge"], np.float32)
    deter_edge = np.asarray(inputs["deter_edge"], np.float32)
    W1 = np.asarray(inputs["W1"], np.float32)
    W2 = np.asarray(inputs["W2"], np.float32)
    W3 = np.asarray(inputs["W3"], np.float32)
    W4 = np.asarray(inputs["W4"], np.float32)
    Wv = np.asarray(inputs["Wv"], np.float32)
    Wout = np.asarray(inputs["Wout"], np.float32)
    b_out = np.asarray(inputs["b_out"], np.float32)
    Wres = np.asarray(inputs["Wres"], np.float32)
    b_res = np.asarray(inputs["b_res"], np.float32)
    ln_g = np.asarray(inputs["ln_g"], np.float32)
    ln_b = np.asarray(inputs["ln_b"], np.float32)

    n_dst = dst_feats.shape[0]
    n_src = src_feats.shape[0]
    E = edge_index.shape[1]

    src_idx = edge_index[0]
    dst_idx = edge_index[1]
    order = np.argsort(dst_idx, kind="stable")
    ds = dst_idx[order]
    ss = src_idx[order]
    Ps = P_edge[order]
    Des = deter_edge[order]

    counts = np.bincount(dst_idx, minlength=n_dst).astype(np.int64)
    cum = np.cumsum(counts)
    bounds = [0]
    for c in range(1, n_cores):
        t = np.searchsorted(cum, E * c // n_cores)
        bounds.append(min(int(t) + 1, n_dst))
    bounds.append(n_dst)
    for i in range(1, len(bounds)):
        bounds[i] = max(bounds[i], bounds[i - 1])

    dstart = np.concatenate([[0], cum]).astype(np.int64)

    def pack(core, KT):
        d0, d1 = bounds[core], bounds[core + 1]
        blocks = []
        d = d0
        cap = KT * 128
        while d < d1:
            e0 = dstart[d]
            nd = 0
            while d + nd < d1 and nd < 128:
                if counts[d + nd] > cap:
                    raise ValueError("dst too big")
                if dstart[d + nd + 1] - e0 > cap:
                    break
                nd += 1
            if nd == 0:
                raise ValueError("cannot place dst")
            blocks.append((int(d), int(nd)))
            d += nd
        return blocks

    best = None
    for KT in KT_candidates:
        try:
            bl = [pack(c, KT) for c in range(n_cores)]
        except ValueError:
            continue
        NB = max(len(b) for b in bl)
        cost = NB * KT
        if best is None or cost < best[0]:
            best = (cost, KT, NB, bl)
    assert best is not None, "no feasible KT"
    _, KT, NB, blocks_per_core = best

    NT = NB * KT
    cfg = Cfg(n_cores=n_cores, n_dst=n_dst, n_src=n_src, NB=NB, KT=KT)

    # host-side constant folding of weights (tiny, weight-only)
    c8 = (W4 @ W3[:, 0]).astype(np.float32)
    wcat2 = np.zeros((2 * D, C), np.float32)
    wcat2[:D, :D] = Wv.T[:, VPERM]
    wcat2[:D, D:] = (W4 @ W2).T
    wcat2[D:, D:] = (W4 @ W1).T
    wor = np.concatenate([Wout.T[VPERM, :], Wres.T], axis=0)  # [128, 64]
    bias64 = (b_out + b_res).astype(np.float32)
    iota128 = np.broadcast_to(np.arange(128, dtype=np.float32)[None, :],
                              (128, 128))

    in_maps = []
    for c in range(n_cores):
        bl = blocks_per_core[c]
        g_src = np.zeros((NT, 128), np.int32)
        g_dstg = np.zeros((NT, 128), np.int32)
        g_slot = np.zeros((NT, 128), np.float32)
        g_P = np.zeros((NT, 128), np.float32)
        g_det = np.full((NT, 128), PAD_DETER, np.float32)
        pad_mask = np.ones((NT, 128), bool)
        for b, (dst0, nd) in enumerate(bl):
            e0 = int(dstart[dst0])
            e1 = int(dstart[dst0 + nd]) if nd > 0 else e0
            ne = e1 - e0
            t0 = b * KT
            j = np.arange(ne)
            tt = t0 + j // 128
            pp = j % 128
            g_src[tt, pp] = ss[e0:e1]
            g_dstg[tt, pp] = ds[e0:e1]
            g_slot[tt, pp] = (ds[e0:e1] - dst0).astype(np.float32)
            g_P[tt, pp] = Ps[e0:e1]
            g_det[tt, pp] = Des[e0:e1]
            pad_mask[tt, pp] = False

        # per-edge concatenated features, transposed: [128, NT*128] bf16
        xs = src_feats[g_src.ravel()]          # [NT*128, 64]
        xd = dst_feats[g_dstg.ravel()]         # [NT*128, 64]
        pm = pad_mask.ravel()
        xs[pm] = 0.0
        xd[pm] = 0.0
        xcatT = _to_bf16(np.concatenate([xs, xd], axis=1).T)  # [128, NT*128]
        xcatT = np.ascontiguousarray(xcatT)

        # slot/P/det as column-duplicated bf16: spd[:, t*6 + {0,1;2,3;4,5}]
        spd = np.empty((128, NT, 6), np.float32)
        spd[:, :, 0] = spd[:, :, 1] = g_slot.T
        spd[:, :, 2] = spd[:, :, 3] = g_P.T
        spd[:, :, 4] = spd[:, :, 5] = g_det.T
        spd = _to_bf16(spd.reshape(128, NT * 6))

        # per-block dst features, transposed: [64, NB*128]
        dslice = np.zeros((NB * 128, D), np.float32)
        for b, (dst0, nd) in enumerate(bl):
            if nd > 0:
                dslice[b * 128: b * 128 + nd] = dst_feats[dst0: dst0 + nd]
        xdstT = _to_bf16(np.ascontiguousarray(dslice.T))

        m = {
            "xcatT": xcatT,
            "spd": np.ascontiguousarray(spd),
            "xdstT": xdstT,
            "iota128": _to_bf16(iota128),
            "wcat2": _to_bf16(wcat2),
            "wor": _to_bf16(wor),
            "c8": _to_bf16(c8.reshape(1, NH)),
            "bias64": np.ascontiguousarray(bias64.reshape(1, D)),
            "ln_g": np.ascontiguousarray(ln_g.reshape(1, D)),
            "ln_b": np.ascontiguousarray(ln_b.reshape(1, D)),
        }
        in_maps.append(m)

    return HostData(cfg=cfg, in_maps=in_maps, blocks=blocks_per_core)


def build_program(cfg: Cfg, debug=False):
    NB, KT = cfg.NB, cfg.KT
    NT = NB * KT

    nc = bacc.Bacc("TRN2", target_bir_lowering=False, debug=debug,
                   num_devices=cfg.n_cores)

    xcatT_d = nc.dram_tensor("xcatT", [128, NT * 128], BF16, kind="ExternalInput")
    spd_d = nc.dram_tensor("spd", [128, NT * 6], BF16, kind="ExternalInput")
    xdstT_d = nc.dram_tensor("xdstT", [D, NB * 128], BF16, kind="ExternalInput")
    iota_d = nc.dram_tensor("iota128", [128, 128], BF16, kind="ExternalInput")
    wcat2_d = nc.dram_tensor("wcat2", [2 * D, C], BF16, kind="ExternalInput")
    wor_d = nc.dram_tensor("wor", [2 * D, D], BF16, kind="ExternalInput")
    c8_d = nc.dram_tensor("c8", [1, NH], BF16, kind="ExternalInput")
    bias64_d = nc.dram_tensor("bias64", [1, D], F32, kind="ExternalInput")
    ln_g_d = nc.dram_tensor("ln_g", [1, D], F32, kind="ExternalInput")
    ln_b_d = nc.dram_tensor("ln_b", [1, D], F32, kind="ExternalInput")

    staged = nc.dram_tensor("staged", [NB * 128, D], F32, kind="ExternalOutput")

    with tile.TileContext(nc) as tc, ExitStack() as ctx:
        consts = ctx.enter_context(tc.tile_pool(name="consts", bufs=1))
        pch = ctx.enter_context(tc.tile_pool(name="pch", bufs=2))
        pblk = ctx.enter_context(tc.tile_pool(name="pblk", bufs=2))
        pout = ctx.enter_context(tc.tile_pool(name="pout", bufs=2))
        psum2 = ctx.enter_context(tc.tile_pool(name="psum2", bufs=2, space="PSUM"))
        psum1 = ctx.enter_context(tc.tile_pool(name="psum1", bufs=1, space="PSUM"))

        ident_b = consts.tile([128, 128], BF16, tag="ident_b")
        make_identity(nc, ident_b[:])
        iota_b = consts.tile([128, 128], BF16, tag="iota_b")
        nc.sync.dma_start(out=iota_b[:], in_=iota_d[:, :])

        c_rep = consts.tile([128, NH], BF16, tag="c_rep")
        nc.sync.dma_start(out=c_rep[:], in_=c8_d[:, :].to_broadcast([128, NH]))
        bias_rep = consts.tile([128, D], F32, tag="bias_rep")
        nc.sync.dma_start(out=bias_rep[:], in_=bias64_d[:, :].to_broadcast([128, D]))
        g_rep = consts.tile([128, D], F32, tag="g_rep")
        nc.sync.dma_start(out=g_rep[:], in_=ln_g_d[:, :].to_broadcast([128, D]))
        b_rep = consts.tile([128, D], F32, tag="b_rep")
        nc.sync.dma_start(out=b_rep[:], in_=ln_b_d[:, :].to_broadcast([128, D]))
        eps_t = consts.tile([128, 1], F32, tag="eps_t")
        nc.vector.memset(eps_t[:], LN_EPS)

        wcat2 = consts.tile([2 * D, C], BF16, tag="wcat2")
        nc.sync.dma_start(out=wcat2[:], in_=wcat2_d[:, :])
        wor_t = consts.tile([2 * D, D], BF16, tag="wor_t")
        nc.sync.dma_start(out=wor_t[:], in_=wor_d[:, :])

        # concat tile: top half gets per-group aggT, bottom half xdstT (static)
        catXD = consts.tile([128, NB * 128], BF16, tag="catXD")
        nc.sync.dma_start(out=catXD[D:2 * D, :], in_=xdstT_d[:, :])

        n_chunks = (NT + CT - 1) // CT
        agg_state = {}
        ln_pend = []          # [(g0b, W), ...] groups awaiting LN flush
        resb_holder = [None]

        def flush_ln():
            b0 = ln_pend[0][0]
            WT = sum(w for _, w in ln_pend)
            ln_pend.clear()
            resb = resb_holder[0]
            WD = WT * D
            mean = pblk.tile([128, GLN], F32, tag="mean")
            nc.vector.tensor_reduce(
                out=mean[:, :WT],
                in_=resb[:, :WD].rearrange("p (g d) -> p g d", g=WT),
                axis=mybir.AxisListType.X, op=ALU.add)
            nc.vector.tensor_scalar_mul(mean[:, :WT], mean[:, :WT], 1.0 / D)
            sq = pout.tile([128, GLN * D], F32, tag="sq")
            nc.gpsimd.tensor_tensor(sq[:, :WD], resb[:, :WD], resb[:, :WD],
                                    op=ALU.mult)
            var = pblk.tile([128, GLN], F32, tag="var")
            nc.vector.tensor_reduce(
                out=var[:, :WT],
                in_=sq[:, :WD].rearrange("p (g d) -> p g d", g=WT),
                axis=mybir.AxisListType.X, op=ALU.add)
            msq = pblk.tile([128, GLN], F32, tag="msq")
            nc.gpsimd.tensor_tensor(msq[:, :WT], mean[:, :WT], mean[:, :WT],
                                    op=ALU.mult)
            nc.vector.scalar_tensor_tensor(
                out=var[:, :WT], in0=var[:, :WT], scalar=1.0 / D,
                in1=msq[:, :WT], op0=ALU.mult, op1=ALU.subtract)
            std = pblk.tile([128, GLN], F32, tag="std")
            nc.scalar.activation(std[:, :WT], var[:, :WT], AF.Sqrt,
                                 bias=eps_t[:, :1])
            nc.vector.reciprocal(std[:, :WT], std[:, :WT])
            xm = pout.tile([128, GLN * D], F32, tag="xm")
            nc.vector.tensor_tensor(xm[:, :WD], resb[:, :WD],
                                    _mkap(mean, [[1, WT], [0, D]]),
                                    op=ALU.subtract)
            nc.vector.tensor_tensor(xm[:, :WD], xm[:, :WD],
                                    _mkap(std, [[1, WT], [0, D]]), op=ALU.mult)
            gb = pout.tile([128, GLN * D], F32, tag="gb")
            nc.gpsimd.tensor_tensor(gb[:, :WD], xm[:, :WD],
                                    _mkap(g_rep, [[0, WT], [1, D]]),
                                    op=ALU.mult)
            nc.vector.tensor_tensor(gb[:, :WD], gb[:, :WD],
                                    _mkap(b_rep, [[0, WT], [1, D]]), op=ALU.add)
            nc.sync.dma_start(
                out=staged[b0 * 128: (b0 + WT) * 128, :].rearrange(
                    "(g p) d -> p g d", p=128),
                in_=gb[:, :WD])

        def group_end(g, agg_ps, W):
            g0b = g * G
            # per-head sums -> reciprocal (f32)
            rcs = pblk.tile([128, G * NH], F32, tag="rcs")
            nc.vector.tensor_scalar_add(
                rcs[:, :W * NH],
                _mkap(agg_ps, [[128, W], [1, NH]], extra_offset=D), 1e-12)
            nc.vector.reciprocal(rcs[:, :W * NH], rcs[:, :W * NH])
            # normalized agg (bf16), V channels are (hd,h)-major so the
            # divisor broadcast has a contiguous inner dim
            aggn = pblk.tile([128, G * D], BF16, tag="aggn")
            nc.vector.tensor_tensor(
                out=_mkap(aggn, [[D, W], [NH, NH], [1, NH]]),
                in0=_mkap(agg_ps, [[128, W], [NH, NH], [1, NH]]),
                in1=_mkap(rcs, [[NH, W], [0, NH], [1, NH]]),
                op=ALU.mult)
            # transpose each block into the concat tile's top half
            at_ps = psum1.tile([D, G * 128], BF16, tag="at", name=f"at{g}")
            for i in range(W):
                nc.tensor.transpose(
                    out=at_ps[:, i * 128: (i + 1) * 128],
                    in_=aggn[:, i * D: (i + 1) * D],
                    identity=ident_b[:])
            nc.vector.tensor_copy(
                catXD[:D, g0b * 128: (g0b + W) * 128], at_ps[:, : W * 128])
            # res = [aggT; xdstT].T @ [Wout_p; Wres]  (one matmul per block)
            res_ps = psum1.tile([128, 512], F32, tag="res", name=f"res{g}")
            for i in range(W):
                nc.tensor.matmul(
                    out=res_ps[:, i * D: (i + 1) * D],
                    lhsT=catXD[:, (g0b + i) * 128: (g0b + i + 1) * 128],
                    rhs=wor_t[:], start=True, stop=True)
            # accumulate biased res rows into the LN staging buffer
            if not ln_pend:
                resb_holder[0] = pout.tile([128, GLN * D], F32, tag="resbacc",
                                           name=f"resbacc{g}")
            off = sum(w for _, w in ln_pend) * D
            nc.vector.tensor_tensor(
                out=resb_holder[0][:, off: off + W * D],
                in0=res_ps[:, : W * D],
                in1=_mkap(bias_rep, [[0, W], [1, D]]), op=ALU.add)
            ln_pend.append((g0b, W))
            if sum(w for _, w in ln_pend) + G > GLN or g0b + W == NB:
                flush_ln()

        for ci in range(n_chunks):
            t0 = ci * CT
            tl = min(CT, NT - t0)

            xc = pch.tile([128, CT * 128], BF16, tag="xc")
            nc.sync.dma_start(xc[:, : tl * 128],
                              xcatT_d[:, t0 * 128: (t0 + tl) * 128])
            spd = pch.tile([128, CT * 6], BF16, tag="spd")
            nc.sync.dma_start(spd[:, : tl * 6],
                              spd_d[:, t0 * 6: (t0 + tl) * 6])

            te_sb = pch.tile([128, CT * C], BF16, tag="teV")

            ng = (tl + TEG - 1) // TEG
            for g0 in range(ng):
                k0 = g0 * TEG
                kn = min(TEG, tl - k0)
                te_ps = psum2.tile([128, 512], F32, tag="te",
                                   name=f"te{ci}_{g0}")
                for kk in range(kn):
                    j = k0 + kk
                    nc.tensor.matmul(
                        out=te_ps[:, kk * C: (kk + 1) * C],
                        lhsT=xc[:, j * 128: (j + 1) * 128],
                        rhs=wcat2[:], start=True, stop=True)
                nc.scalar.copy(
                    out=te_sb[:, k0 * C: (k0 + kn) * C],
                    in_=te_ps[:, : kn * C])

            # logits: l = P*c8 + det + A ; leaky(0.2)  (bf16, on gpsimd)
            lbuf = pch.tile([128, CT * NH], BF16, tag="lbuf")
            nc.gpsimd.tensor_tensor(
                out=_mkap(lbuf, [[NH, tl], [2, 4], [1, 2]]),
                in0=_mkap(spd, [[6, tl], [0, 4], [1, 2]], extra_offset=2),
                in1=_mkap(c_rep, [[0, tl], [2, 4], [1, 2]]), op=ALU.mult)
            nc.gpsimd.tensor_tensor(
                out=_mkap(lbuf, [[NH, tl], [2, 4], [1, 2]]),
                in0=_mkap(lbuf, [[NH, tl], [2, 4], [1, 2]]),
                in1=_mkap(spd, [[6, tl], [0, 4], [1, 2]], extra_offset=4),
                op=ALU.add)
            nc.gpsimd.tensor_tensor(
                out=_mkap(lbuf, [[NH, tl], [2, 4], [1, 2]]),
                in0=_mkap(lbuf, [[NH, tl], [2, 4], [1, 2]]),
                in1=_mkap(te_sb, [[C, tl], [2, 4], [1, 2]], extra_offset=D),
                op=ALU.add)
            nc.vector.scalar_tensor_tensor(
                out=lbuf[:, : tl * NH], in0=lbuf[:, : tl * NH], scalar=0.2,
                in1=lbuf[:, : tl * NH], op0=ALU.mult, op1=ALU.max)

            rhs_buf = pch.tile([128, CT * C], BF16, tag="rhs")
            nc.scalar.activation(
                _mkap(rhs_buf, [[C, tl], [1, NH]], extra_offset=D),
                lbuf[:, : tl * NH], AF.Exp)
            # V * expl (V channels (hd,h)-major -> contiguous bcast inner dim)
            nc.vector.tensor_tensor(
                out=_mkap(rhs_buf, [[C, tl], [NH, NH], [1, NH]]),
                in0=_mkap(te_sb, [[C, tl], [NH, NH], [1, NH]]),
                in1=_mkap(rhs_buf, [[C, tl], [0, NH], [1, NH]],
                          extra_offset=D),
                op=ALU.mult)

            # one-hot H via slot-dup pairs (2x eligible)
            H_all = pch.tile([128, CT * 128], BF16, tag="H")
            nc.vector.tensor_tensor(
                out=_mkap(H_all, [[128, tl], [2, 64], [1, 2]]),
                in0=_mkap(spd, [[6, tl], [0, 64], [1, 2]]),
                in1=_mkap(iota_b, [[0, tl], [2, 64], [1, 2]]),
                op=ALU.is_equal)

            for j in range(tl):
                t = t0 + j
                b, k = divmod(t, KT)
                g, bi = divmod(b, G)
                if k == 0 and bi == 0:
                    agg_state[g] = psum2.tile([128, G * 128], F32, tag="agg",
                                              name=f"agg{g}")
                nc.tensor.matmul(
                    out=agg_state[g][:, bi * 128: bi * 128 + C],
                    lhsT=H_all[:, j * 128: (j + 1) * 128],
                    rhs=rhs_buf[:, j * C: (j + 1) * C],
                    start=(k == 0), stop=(k == KT - 1))
                if k == KT - 1 and (bi == G - 1 or b == NB - 1):
                    group_end(g, agg_state.pop(g), bi + 1)

        assert not agg_state, "unflushed agg group"
        assert not ln_pend, "unflushed LN group"

    nc.compile()
    return nc


def unpack_output(hd: HostData, results):
    cfg = hd.cfg
    out = np.zeros((cfg.n_dst, D), np.float32)
    for c in range(cfg.n_cores):
        staged = results[c]["staged"]
        for b, (dst0, nd) in enumerate(hd.blocks[c]):
            if nd > 0:
                out[dst0: dst0 + nd] = staged[b * 128: b * 128 + nd]
    return out


_prog_cache = {}


def kernel(**inputs) -> np.ndarray:
    from concourse.bass_utils import run_bass_kernel_spmd

    hd = prepare_host(inputs, n_cores=8)
    key = (hd.cfg.NB, hd.cfg.KT)
    if key not in _prog_cache:
        _prog_cache[key] = build_program(hd.cfg)
    nc = _prog_cache[key]
    res = run_bass_kernel_spmd(
        nc, hd.in_maps, core_ids=list(range(hd.cfg.n_cores)),
        trace=bool(int(os.environ.get("KERNEL_TRACE", "0"))),
    )
    out = unpack_output(hd, res.results)
    if res.exec_time_ns is not None:
        print(f"HW exec time: {res.exec_time_ns} ns", file=sys.stderr)
        kernel.last_exec_time_ns = res.exec_time_ns
    return out


kernel.last_exec_time_ns = None
